# revision 22
# baseline (speedup 1.0000x reference)
"""COLoRALinear fused kernel for 8 TRN2 NeuronCores (Bass/Tile).

Computation (per reference):
  base_out   = x @ W^T + b                         [B,S,Do]
  shared_out = (x @ As^T) @ Bs^T * SCALING
  routing    = softmax(mean_s(x) @ task_emb^T)     [B,E]
  t          = x @ Ae^T (per expert)               [B,S,E,R]
  task_out   = sum_e routing[b,e] * t_e @ Be^T * SCALING
  out = base_out + cw*shared_out + (1-cw)*task_out,  cw = sigmoid(collab_w)

Sharding: flatten x to [B*S, Din] = [8192, 2048]; core c owns rows
[c*1024, (c+1)*1024) — all from batch b = c//2.  W and the low-rank
params are replicated.  The routing mean needs all of batch b, so each
core reduces its half and a pairwise AllReduce ([[0,1],[2,3],...])
completes the per-batch logits on-device.

On-core algorithm (all matmuls bf16 with fp32 PSUM accumulation):
  stage1: u[80, m] = Aall @ x_shard^T where Aall stacks
          [shared_A (8); expert_A (64); task_emb/S (8)].
  logits: rows 72:80 of u, reduced over m, pair-AllReduced -> softmax.
  scale:  per-row scales for u via a tiny matmul with Emap (folds
          (1-cw)*SCALING*r_e for expert rows, 1.0 for shared rows since
          sum_e r_e = 1; cw*SCALING is folded into C2's shared rows).
  stage2: out_chunk += u_scaled^T @ C2 as the 17th accumulating matmul
          on top of 16 base-matmul K-chunks; C2 row 72 = base_b with a
          ones-row in u_scaled providing the bias.
"""

import numpy as np
import ml_dtypes

import concourse.bass as bass
import concourse.mybir as mybir
import concourse.tile as tile
from concourse import bacc
from concourse.bass import ts
from concourse.bass_utils import run_bass_kernel_spmd

# Problem shapes (hardcoded per spec)
B, S, DIN, DOUT = 4, 2048, 2048, 2048
E, R = 8, 8
SCALING = 16.0 / 8.0
N_CORES = 8
M_CORE = B * S // N_CORES          # 1024 rows per core
P = 128                            # partitions
KT = DIN // P                      # 16 contraction chunks
NOC = DOUT // 512                  # 4 output chunks of 512
NMT = M_CORE // P                  # 8 m-tiles of 128
AW = 80                            # rows of A-stack: 8 taskemb + 8 shared + 64 expert
CW = 81                            # rows of C2: 8 zero + 8 shared + 64 expert + 1 bias
DEFER = 24                         # chunks staged to SBUF before routing is ready
WQ = 4                             # WT slab split: KT/WQ i-chunks per DMA
WARMUP_MM = 40                     # junk matmuls to flip the PE HAM clock-gate early

BF16 = ml_dtypes.bfloat16

# set by test.py for profiling
TRACE = False
LAST_RESULT = None

_cached = None


def _build_nc():
    nc = bacc.Bacc(
        "TRN2",
        target_bir_lowering=False,
        debug=False,
        num_devices=N_CORES,
    )
    BF = mybir.dt.bfloat16
    F32 = mybir.dt.float32

    # host-packed layouts: partition-major so every DMA reads large
    # contiguous runs per partition
    xT_d = nc.dram_tensor("xT", [DIN, M_CORE], BF, kind="ExternalInput")
    WT_d = nc.dram_tensor("WT", [P, NOC, KT, 512], BF, kind="ExternalInput")
    AallT_d = nc.dram_tensor("AallT", [P, KT, AW], BF, kind="ExternalInput")
    C2_d = nc.dram_tensor("C2", [CW, DOUT], BF, kind="ExternalInput")
    Emap_d = nc.dram_tensor("Emap", [E, AW], BF, kind="ExternalInput")
    out_d = nc.dram_tensor("out", [M_CORE, DOUT], F32, kind="ExternalOutput")

    ones_d = nc.dram_tensor("ones", [M_CORE], BF, kind="ExternalInput")

    cc_in = nc.dram_tensor("cc_in", [E], F32)
    cc_out = nc.dram_tensor("cc_out", [E], F32)
    r_bounce = nc.dram_tensor("r_bounce", [E], BF)

    X = mybir.AxisListType.X

    with tile.TileContext(nc) as tc:
        with (
            tc.tile_pool(name="consts", bufs=1) as consts,
            tc.tile_pool(name="small", bufs=1) as small,
            tc.tile_pool(name="pmm", bufs=6, space="PSUM") as pmm,
            tc.tile_pool(name="psmall", bufs=1, space="PSUM") as psmall,
            tc.tile_pool(name="outp", bufs=3) as outp,
        ):
            # ---- constant / input loads ----
            # One FIFO HW queue services all sync-engine DMAs, so issue
            # order == arrival order.  Interleave xT with WT's first slab so
            # the base loop can start right after stage-1 drains.
            AallT_sb = consts.tile([P, KT, AW], BF)
            nc.sync.dma_start(AallT_sb[:, :, :], AallT_d[:, :, :])
            xT_sb = consts.tile([P, KT, M_CORE], BF)
            WT_sb = consts.tile([P, NOC, KT, 512], BF)

            def wt_load(oc, iq):
                nc.sync.dma_start(
                    WT_sb[:, oc, iq * WQ : (iq + 1) * WQ, :],
                    WT_d[:, oc, iq * WQ : (iq + 1) * WQ, :],
                )

            for i in range(0, 8):
                nc.sync.dma_start(xT_sb[:, i, :], xT_d[ts(i, P), :])
            wt_load(0, 0)
            wt_load(0, 1)
            for i in range(8, KT):
                nc.sync.dma_start(xT_sb[:, i, :], xT_d[ts(i, P), :])
            wt_load(0, 2)
            wt_load(0, 3)
            Emap_sb = consts.tile([E, AW], BF)
            nc.sync.dma_start(Emap_sb[:], Emap_d[:, :])
            C2_sb = consts.tile([CW, DOUT], BF)
            nc.sync.dma_start(C2_sb[:], C2_d[:, :])
            for oc in range(1, NOC):
                for iq in range(KT // WQ):
                    wt_load(oc, iq)

            # ---- PE warmup ----
            # Depends only on the first (small) AallT DMA; keeps the PE busy
            # before stage-1 so the HAM clock-gate reaches 2.4GHz early.
            # Results are never read.
            warm_ps = pmm.tile([P, 512], mybir.dt.float32, tag="ps")

            def junk_mm(w):
                nc.tensor.matmul(
                    warm_ps[0:AW, 0:AW],
                    AallT_sb[:, w % KT, :],
                    AallT_sb[:, (w * 7 + 3) % KT, :],
                    start=True,
                    stop=True,
                )

            for w in range(WARMUP_MM):
                junk_mm(w)

            # ---- stage 1: u[80, m], both m-halves interleaved per i so the
            # PE duty cycle stays high while xT tiles stream in (HAM stays
            # warm); junk fillers plug the remaining DMA-pacing gaps ----
            # A-stack rows: 0..7 taskemb, 8..15 shared, 16..79 expert
            u_sb = small.tile([AW, M_CORE], F32)
            u_ps_a = psmall.tile([AW, 512], mybir.dt.float32, tag="u_ps")
            u_ps_b = psmall.tile([AW, 512], mybir.dt.float32, tag="scale_ps")
            u_ps = {0: u_ps_a, 1: u_ps_b}
            for i in range(KT):
                for h in range(2):
                    nc.tensor.matmul(
                        u_ps[h][:, :],
                        AallT_sb[:, i, :],
                        xT_sb[:, i, ts(h, 512)],
                        start=(i == 0),
                        stop=(i == KT - 1),
                    )
                junk_mm(2 * i)
                junk_mm(2 * i + 1)
            lg_parts = []
            for h in range(2):
                lg_h = small.tile([E, 1], F32, tag=f"lg{h}")
                nc.vector.reduce_sum(lg_h[0:8, :], u_ps[h][0:8, :], axis=X)
                nc.vector.tensor_copy(u_sb[:, ts(h, 512)], u_ps[h][0:AW, :])
                lg_parts.append(lg_h)
            lg = small.tile([E, 1], F32, tag="lg")
            nc.vector.tensor_add(
                lg[0:8, :], lg_parts[0][0:8, :], lg_parts[1][0:8, :]
            )

            # ---- cross-core logits reduction (pairs share a batch) ----
            # control-path DMAs use gpsimd SWDGE: off the bulk HW queue,
            # so they don't wait behind the WT/x loads
            nc.gpsimd.dma_start(cc_in[:], lg[0:8, 0:1])
            nc.gpsimd.collective_compute(
                "AllReduce",
                mybir.AluOpType.add,
                replica_groups=[[0, 1], [2, 3], [4, 5], [6, 7]],
                ins=[cc_in.ap().opt()],
                outs=[cc_out.ap().opt()],
            )

            # ---- softmax over E on one partition ----
            lrow = small.tile([1, E], F32)
            nc.gpsimd.dma_start(lrow[:], cc_out[:])
            mx = small.tile([1, 1], F32)
            nc.vector.reduce_max(mx[:], lrow[:], axis=X)
            shf = small.tile([1, E], F32)
            nc.vector.tensor_scalar_sub(shf[:], lrow[:], mx[0:1, 0:1])
            ex = small.tile([1, E], F32)
            nc.scalar.activation(ex[:], shf[:], mybir.ActivationFunctionType.Exp)
            sm = small.tile([1, 1], F32)
            nc.vector.reduce_sum(sm[:], ex[:], axis=X)
            ri = small.tile([1, 1], F32)
            nc.vector.reciprocal(ri[:], sm[:])
            rrow = small.tile([1, E], BF)
            nc.vector.tensor_scalar_mul(rrow[:], ex[:], ri[0:1, 0:1])
            nc.gpsimd.dma_start(r_bounce[:], rrow[:])
            rcol = small.tile([E, 1], BF)
            nc.gpsimd.dma_start(rcol[:], r_bounce[:])

            u_scaled = small.tile([CW, M_CORE], BF)
            # bias ones-row (row 80) via DMA — engine ops need 32-aligned
            # partition bases, DMA does not
            nc.gpsimd.dma_start(u_scaled[AW : AW + 1, :], ones_d[:])

            def emit_scale_chain():
                scale_ps = psmall.tile([AW, 1], mybir.dt.float32, tag="scale_ps")
                nc.tensor.matmul(
                    scale_ps[:], Emap_sb[:, :], rcol[:], start=True, stop=True
                )
                scale_sb = small.tile([AW, 1], F32)
                nc.vector.tensor_copy(scale_sb[:], scale_ps[:])
                nc.vector.tensor_scalar_mul(
                    u_scaled[0:AW, :], u_sb[0:AW, :], scale_sb[0:AW, 0:1]
                )

            def finish_chunk(mt, oc, ps):
                # 17th accumulating matmul: shared+task low-rank + bias
                nc.tensor.matmul(
                    ps[:],
                    u_scaled[0:CW, ts(mt, P)],
                    C2_sb[0:CW, ts(oc, 512)],
                    start=False,
                    stop=True,
                )
                ob = outp.tile([P, 512], F32, tag="ob")
                nc.vector.tensor_copy(ob[:], ps[:])
                nc.sync.dma_start(out_d[ts(mt, P), ts(oc, 512)], ob[:])

            def finish_deferred(mt, oc, stage_sb):
                # low-rank product into a fresh psum, added to the staged
                # base result on the way out
                ps2 = pmm.tile([P, 512], mybir.dt.float32, tag="ps")
                nc.tensor.matmul(
                    ps2[:],
                    u_scaled[0:CW, ts(mt, P)],
                    C2_sb[0:CW, ts(oc, 512)],
                    start=True,
                    stop=True,
                )
                ob = outp.tile([P, 512], F32, tag="ob")
                nc.vector.tensor_add(ob[:], stage_sb[:], ps2[:])
                nc.sync.dma_start(out_d[ts(mt, P), ts(oc, 512)], ob[:])

            # ---- main loop: base matmul + fused epilogue ----
            # The first DEFER chunks finish with base-only results staged to
            # SBUF; their low-rank term is added once the routing collective
            # has delivered u_scaled.  This keeps the PE stream dense while
            # the (slow, ~20us) collective is in flight, without holding
            # PSUM banks.
            chunk_idx = 0
            deferred = []
            with tc.tile_pool(name="defer", bufs=DEFER) as defer_pool:
                for oc in range(NOC):
                    for mt in range(NMT):
                        ps = pmm.tile([P, 512], mybir.dt.float32, tag="ps")
                        for i in range(KT):
                            nc.tensor.matmul(
                                ps[:],
                                xT_sb[:, i, ts(mt, P)],
                                WT_sb[:, oc, i, :],
                                start=(i == 0),
                                stop=(chunk_idx < DEFER and i == KT - 1),
                            )
                        if chunk_idx < DEFER:
                            stage_sb = defer_pool.tile(
                                [P, 512], F32, tag="stage"
                            )
                            nc.vector.tensor_copy(stage_sb[:], ps[:])
                            deferred.append((mt, oc, stage_sb))
                        else:
                            finish_chunk(mt, oc, ps)
                            # drain deferred chunks gradually so their DVE
                            # adds interleave with ongoing base matmuls
                            for _ in range(3):
                                if deferred:
                                    dmt, doc, dsb = deferred.pop(0)
                                    finish_deferred(dmt, doc, dsb)
                        chunk_idx += 1
                        if chunk_idx == DEFER:
                            emit_scale_chain()
                for dmt, doc, dsb in deferred:
                    finish_deferred(dmt, doc, dsb)

    nc.compile()
    return nc


def _prep_inputs(x, base_W, base_b, shared_A, shared_B, expert_A, expert_B,
                 task_emb, collab_w):
    f = np.float32
    x = np.asarray(x, dtype=f).reshape(B * S, DIN)
    base_W = np.asarray(base_W, dtype=f)
    base_b = np.asarray(base_b, dtype=f)
    shared_A = np.asarray(shared_A, dtype=f)
    shared_B = np.asarray(shared_B, dtype=f)
    expert_A = np.asarray(expert_A, dtype=f)
    expert_B = np.asarray(expert_B, dtype=f)
    task_emb = np.asarray(task_emb, dtype=f)
    cw = float(1.0 / (1.0 + np.exp(-np.asarray(collab_w, dtype=np.float64))))

    # partition-major packed layouts (large contiguous DMA bursts):
    # WT[p, oc, i, j] = base_W.T[i*128+p, oc*512+j]
    WT = np.ascontiguousarray(
        base_W.T.reshape(KT, P, NOC, 512).transpose(1, 2, 0, 3)
    ).astype(BF16)                                                   # [P,NOC,KT,512]
    # A-stack rows: 0..7 taskemb/S (logits), 8..15 shared, 16..79 expert
    A_all = np.concatenate(
        [task_emb / float(S), shared_A, expert_A.reshape(E * R, DIN)], axis=0
    )                                                                # [80, DIN]
    # AallT[p, i, a] = A_all[a, i*128+p]
    AallT = np.ascontiguousarray(
        A_all.T.reshape(KT, P, AW).transpose(1, 0, 2)
    ).astype(BF16)                                                   # [P,KT,AW]

    # C2 rows align with u_scaled rows; row 80 = bias via ones-row
    C2 = np.zeros((CW, DOUT), dtype=f)
    C2[8:16] = shared_B.T * (cw * SCALING)
    C2[16:80] = expert_B.transpose(0, 2, 1).reshape(E * R, DOUT)
    C2[80] = base_b
    C2 = C2.astype(BF16)

    # scale[j] = sum_e Emap[e, j] * r[e]:
    #   taskemb rows -> 0, shared rows -> 1 (softmax sums to 1),
    #   expert row (e,r) -> (1-cw)*SCALING*r_e
    Emap = np.zeros((E, AW), dtype=f)
    Emap[:, 8:16] = 1.0
    for e in range(E):
        Emap[e, 16 + 8 * e : 24 + 8 * e] = (1.0 - cw) * SCALING
    Emap = Emap.astype(BF16)

    ones = np.ones((M_CORE,), dtype=BF16)

    in_maps = []
    for c in range(N_CORES):
        xT = np.ascontiguousarray(x[c * M_CORE : (c + 1) * M_CORE].T).astype(BF16)
        in_maps.append(
            {"xT": xT, "WT": WT, "AallT": AallT, "C2": C2, "Emap": Emap,
             "ones": ones}
        )
    return in_maps


def kernel(**inputs):
    global _cached, LAST_RESULT
    if _cached is None:
        _cached = _build_nc()
    nc = _cached
    in_maps = _prep_inputs(**inputs)
    res = run_bass_kernel_spmd(
        nc, in_maps, core_ids=list(range(N_CORES)), trace=TRACE
    )
    LAST_RESULT = res
    out = np.concatenate(
        [res.results[c]["out"] for c in range(N_CORES)], axis=0
    ).reshape(B, S, DOUT)
    return np.ascontiguousarray(out.astype(np.float32))


# revision 24
# speedup vs baseline: 1.0993x; 1.0993x over previous
"""COLoRALinear fused kernel for 8 TRN2 NeuronCores (Bass/Tile).

Computation (per reference):
  base_out   = x @ W^T + b                         [B,S,Do]
  shared_out = (x @ As^T) @ Bs^T * SCALING
  routing    = softmax(mean_s(x) @ task_emb^T)     [B,E]
  t          = x @ Ae^T (per expert)               [B,S,E,R]
  task_out   = sum_e routing[b,e] * t_e @ Be^T * SCALING
  out = base_out + cw*shared_out + (1-cw)*task_out,  cw = sigmoid(collab_w)

Sharding: flatten x to [B*S, Din] = [8192, 2048]; core c owns rows
[c*1024, (c+1)*1024) — all from batch b = c//2.  W and the low-rank
params are replicated.  The routing mean needs all of batch b, so each
core reduces its half and a pairwise AllReduce ([[0,1],[2,3],...])
completes the per-batch logits on-device.

On-core algorithm (all matmuls bf16 with fp32 PSUM accumulation):
  stage1: u[80, m] = Aall @ x_shard^T where Aall stacks
          [shared_A (8); expert_A (64); task_emb/S (8)].
  logits: rows 72:80 of u, reduced over m, pair-AllReduced -> softmax.
  scale:  per-row scales for u via a tiny matmul with Emap (folds
          (1-cw)*SCALING*r_e for expert rows, 1.0 for shared rows since
          sum_e r_e = 1; cw*SCALING is folded into C2's shared rows).
  stage2: out_chunk += u_scaled^T @ C2 as the 17th accumulating matmul
          on top of 16 base-matmul K-chunks; C2 row 72 = base_b with a
          ones-row in u_scaled providing the bias.
"""

import numpy as np
import ml_dtypes

import concourse.bass as bass
import concourse.mybir as mybir
import concourse.tile as tile
from concourse import bacc
from concourse.bass import ts
from concourse.bass_utils import run_bass_kernel_spmd

# Problem shapes (hardcoded per spec)
B, S, DIN, DOUT = 4, 2048, 2048, 2048
E, R = 8, 8
SCALING = 16.0 / 8.0
N_CORES = 8
M_CORE = B * S // N_CORES          # 1024 rows per core
P = 128                            # partitions
KT = DIN // P                      # 16 contraction chunks
NOC = DOUT // 512                  # 4 output chunks of 512
NMT = M_CORE // P                  # 8 m-tiles of 128
AW = 80                            # rows of A-stack: 8 taskemb + 8 shared + 64 expert
CW = 81                            # rows of C2: 8 zero + 8 shared + 64 expert + 1 bias
DEFER = 20                         # chunks staged to SBUF before routing is ready
WQ = 4                             # WT slab split: KT/WQ i-chunks per DMA
WARMUP_MM = 40                     # junk matmuls to flip the PE HAM clock-gate early

BF16 = ml_dtypes.bfloat16

# set by test.py for profiling
TRACE = False
LAST_RESULT = None

_cached = None


def _build_nc():
    nc = bacc.Bacc(
        "TRN2",
        target_bir_lowering=False,
        debug=False,
        num_devices=N_CORES,
    )
    BF = mybir.dt.bfloat16
    F32 = mybir.dt.float32

    # host-packed layouts: partition-major so every DMA reads large
    # contiguous runs per partition
    xT_d = nc.dram_tensor("xT", [DIN, M_CORE], BF, kind="ExternalInput")
    WT_d = nc.dram_tensor("WT", [P, NOC, KT, 512], BF, kind="ExternalInput")
    AallT_d = nc.dram_tensor("AallT", [P, KT, AW], BF, kind="ExternalInput")
    C2_d = nc.dram_tensor("C2", [CW, DOUT], BF, kind="ExternalInput")
    Emap_d = nc.dram_tensor("Emap", [E, AW], BF, kind="ExternalInput")
    out_d = nc.dram_tensor("out", [M_CORE, DOUT], F32, kind="ExternalOutput")

    ones_d = nc.dram_tensor("ones", [M_CORE], BF, kind="ExternalInput")

    cc_in = nc.dram_tensor("cc_in", [E], F32)
    cc_out = nc.dram_tensor("cc_out", [E], F32)
    r_bounce = nc.dram_tensor("r_bounce", [E], BF)

    X = mybir.AxisListType.X

    with tile.TileContext(nc) as tc:
        with (
            tc.tile_pool(name="consts", bufs=1) as consts,
            tc.tile_pool(name="small", bufs=1) as small,
            tc.tile_pool(name="pmm", bufs=6, space="PSUM") as pmm,
            tc.tile_pool(name="psmall", bufs=1, space="PSUM") as psmall,
            tc.tile_pool(name="outp", bufs=3) as outp,
        ):
            # ---- constant / input loads ----
            # One FIFO HW queue services all sync-engine DMAs, so issue
            # order == arrival order.  Interleave xT with WT's first slab so
            # the base loop can start right after stage-1 drains.
            AallT_sb = consts.tile([P, KT, AW], BF)
            nc.sync.dma_start(AallT_sb[:, :, :], AallT_d[:, :, :])
            xT_sb = consts.tile([P, KT, M_CORE], BF)
            WT_sb = consts.tile([P, NOC, KT, 512], BF)

            def wt_load(oc, iq):
                nc.sync.dma_start(
                    WT_sb[:, oc, iq * WQ : (iq + 1) * WQ, :],
                    WT_d[:, oc, iq * WQ : (iq + 1) * WQ, :],
                )

            for i in range(0, 8):
                nc.sync.dma_start(xT_sb[:, i, :], xT_d[ts(i, P), :])
            wt_load(0, 0)
            wt_load(0, 1)
            for i in range(8, KT):
                nc.sync.dma_start(xT_sb[:, i, :], xT_d[ts(i, P), :])
            wt_load(0, 2)
            wt_load(0, 3)
            Emap_sb = consts.tile([E, AW], BF)
            nc.sync.dma_start(Emap_sb[:], Emap_d[:, :])
            C2_sb = consts.tile([CW, DOUT], BF)
            nc.sync.dma_start(C2_sb[:], C2_d[:, :])
            for oc in range(1, NOC):
                for iq in range(KT // WQ):
                    wt_load(oc, iq)

            # ---- PE warmup ----
            # Depends only on the first (small) AallT DMA; keeps the PE busy
            # before stage-1 so the HAM clock-gate reaches 2.4GHz early.
            # Results are never read.
            warm_ps = pmm.tile([P, 512], mybir.dt.float32, tag="ps")

            def junk_mm(w):
                nc.tensor.matmul(
                    warm_ps[0:AW, 0:AW],
                    AallT_sb[:, w % KT, :],
                    AallT_sb[:, (w * 7 + 3) % KT, :],
                    start=True,
                    stop=True,
                )

            for w in range(WARMUP_MM):
                junk_mm(w)

            # ---- stage 1: u[80, m], both m-halves interleaved per i so the
            # PE duty cycle stays high while xT tiles stream in (HAM stays
            # warm); junk fillers plug the remaining DMA-pacing gaps ----
            # A-stack rows: 0..7 taskemb, 8..15 shared, 16..79 expert
            u_sb = small.tile([AW, M_CORE], F32)
            u_ps_a = psmall.tile([AW, 512], mybir.dt.float32, tag="u_ps")
            u_ps_b = psmall.tile([AW, 512], mybir.dt.float32, tag="scale_ps")
            u_ps = {0: u_ps_a, 1: u_ps_b}
            for i in range(KT):
                for h in range(2):
                    nc.tensor.matmul(
                        u_ps[h][:, :],
                        AallT_sb[:, i, :],
                        xT_sb[:, i, ts(h, 512)],
                        start=(i == 0),
                        stop=(i == KT - 1),
                    )
                junk_mm(2 * i)
                junk_mm(2 * i + 1)
            lg_parts = []
            for h in range(2):
                lg_h = small.tile([E, 1], F32, tag=f"lg{h}")
                nc.vector.reduce_sum(lg_h[0:8, :], u_ps[h][0:8, :], axis=X)
                nc.vector.tensor_copy(u_sb[:, ts(h, 512)], u_ps[h][0:AW, :])
                lg_parts.append(lg_h)
            lg = small.tile([E, 1], F32, tag="lg")
            nc.vector.tensor_add(
                lg[0:8, :], lg_parts[0][0:8, :], lg_parts[1][0:8, :]
            )

            # ---- cross-core logits reduction (pairs share a batch) ----
            # control-path DMAs use gpsimd SWDGE: off the bulk HW queue,
            # so they don't wait behind the WT/x loads
            nc.gpsimd.dma_start(cc_in[:], lg[0:8, 0:1])
            nc.gpsimd.collective_compute(
                "AllReduce",
                mybir.AluOpType.add,
                replica_groups=[[0, 1], [2, 3], [4, 5], [6, 7]],
                ins=[cc_in.ap().opt()],
                outs=[cc_out.ap().opt()],
            )

            # ---- softmax over E on one partition ----
            lrow = small.tile([1, E], F32)
            nc.gpsimd.dma_start(lrow[:], cc_out[:])
            mx = small.tile([1, 1], F32)
            nc.vector.reduce_max(mx[:], lrow[:], axis=X)
            shf = small.tile([1, E], F32)
            nc.vector.tensor_scalar_sub(shf[:], lrow[:], mx[0:1, 0:1])
            ex = small.tile([1, E], F32)
            nc.scalar.activation(ex[:], shf[:], mybir.ActivationFunctionType.Exp)
            sm = small.tile([1, 1], F32)
            nc.vector.reduce_sum(sm[:], ex[:], axis=X)
            ri = small.tile([1, 1], F32)
            nc.vector.reciprocal(ri[:], sm[:])
            rrow = small.tile([1, E], BF)
            nc.vector.tensor_scalar_mul(rrow[:], ex[:], ri[0:1, 0:1])
            nc.gpsimd.dma_start(r_bounce[:], rrow[:])
            rcol = small.tile([E, 1], BF)
            nc.gpsimd.dma_start(rcol[:], r_bounce[:])

            u_scaled = small.tile([CW, M_CORE], BF)
            # bias ones-row (row 80) via DMA — engine ops need 32-aligned
            # partition bases, DMA does not
            nc.gpsimd.dma_start(u_scaled[AW : AW + 1, :], ones_d[:])

            def emit_scale_chain():
                scale_ps = psmall.tile([AW, 1], mybir.dt.float32, tag="scale_ps")
                nc.tensor.matmul(
                    scale_ps[:], Emap_sb[:, :], rcol[:], start=True, stop=True
                )
                scale_sb = small.tile([AW, 1], F32)
                nc.vector.tensor_copy(scale_sb[:], scale_ps[:])
                nc.vector.tensor_scalar_mul(
                    u_scaled[0:AW, :], u_sb[0:AW, :], scale_sb[0:AW, 0:1]
                )

            def finish_chunk(mt, oc, ps):
                # 17th accumulating matmul: shared+task low-rank + bias
                nc.tensor.matmul(
                    ps[:],
                    u_scaled[0:CW, ts(mt, P)],
                    C2_sb[0:CW, ts(oc, 512)],
                    start=False,
                    stop=True,
                )
                ob = outp.tile([P, 512], F32, tag="ob")
                nc.vector.tensor_copy(ob[:], ps[:])
                nc.sync.dma_start(out_d[ts(mt, P), ts(oc, 512)], ob[:])

            def finish_deferred(mt, oc, stage_sb):
                # low-rank product into a fresh psum, added to the staged
                # base result on the way out
                ps2 = pmm.tile([P, 512], mybir.dt.float32, tag="ps")
                nc.tensor.matmul(
                    ps2[:],
                    u_scaled[0:CW, ts(mt, P)],
                    C2_sb[0:CW, ts(oc, 512)],
                    start=True,
                    stop=True,
                )
                ob = outp.tile([P, 512], F32, tag="ob")
                nc.vector.tensor_add(ob[:], stage_sb[:], ps2[:])
                nc.sync.dma_start(out_d[ts(mt, P), ts(oc, 512)], ob[:])

            # ---- main loop: base matmul + fused epilogue ----
            # The first DEFER chunks finish with base-only results staged to
            # SBUF; their low-rank term is added once the routing collective
            # has delivered u_scaled.  This keeps the PE stream dense while
            # the (slow, ~20us) collective is in flight, without holding
            # PSUM banks.
            chunk_idx = 0
            deferred = []
            with tc.tile_pool(name="defer", bufs=DEFER) as defer_pool:
                for oc in range(NOC):
                    for mt in range(NMT):
                        ps = pmm.tile([P, 512], mybir.dt.float32, tag="ps")
                        for i in range(KT):
                            nc.tensor.matmul(
                                ps[:],
                                xT_sb[:, i, ts(mt, P)],
                                WT_sb[:, oc, i, :],
                                start=(i == 0),
                                stop=(chunk_idx < DEFER and i == KT - 1),
                            )
                        if chunk_idx < DEFER:
                            stage_sb = defer_pool.tile(
                                [P, 512], F32, tag="stage"
                            )
                            nc.vector.tensor_copy(stage_sb[:], ps[:])
                            deferred.append((mt, oc, stage_sb))
                        else:
                            finish_chunk(mt, oc, ps)
                            # drain deferred chunks gradually so their DVE
                            # adds interleave with ongoing base matmuls
                            for _ in range(2):
                                if deferred:
                                    dmt, doc, dsb = deferred.pop(0)
                                    finish_deferred(dmt, doc, dsb)
                        chunk_idx += 1
                        if chunk_idx == DEFER:
                            emit_scale_chain()
                for dmt, doc, dsb in deferred:
                    finish_deferred(dmt, doc, dsb)

    nc.compile()
    return nc


def _prep_inputs(x, base_W, base_b, shared_A, shared_B, expert_A, expert_B,
                 task_emb, collab_w):
    f = np.float32
    x = np.asarray(x, dtype=f).reshape(B * S, DIN)
    base_W = np.asarray(base_W, dtype=f)
    base_b = np.asarray(base_b, dtype=f)
    shared_A = np.asarray(shared_A, dtype=f)
    shared_B = np.asarray(shared_B, dtype=f)
    expert_A = np.asarray(expert_A, dtype=f)
    expert_B = np.asarray(expert_B, dtype=f)
    task_emb = np.asarray(task_emb, dtype=f)
    cw = float(1.0 / (1.0 + np.exp(-np.asarray(collab_w, dtype=np.float64))))

    # partition-major packed layouts (large contiguous DMA bursts):
    # WT[p, oc, i, j] = base_W.T[i*128+p, oc*512+j]
    WT = np.ascontiguousarray(
        base_W.T.reshape(KT, P, NOC, 512).transpose(1, 2, 0, 3)
    ).astype(BF16)                                                   # [P,NOC,KT,512]
    # A-stack rows: 0..7 taskemb/S (logits), 8..15 shared, 16..79 expert
    A_all = np.concatenate(
        [task_emb / float(S), shared_A, expert_A.reshape(E * R, DIN)], axis=0
    )                                                                # [80, DIN]
    # AallT[p, i, a] = A_all[a, i*128+p]
    AallT = np.ascontiguousarray(
        A_all.T.reshape(KT, P, AW).transpose(1, 0, 2)
    ).astype(BF16)                                                   # [P,KT,AW]

    # C2 rows align with u_scaled rows; row 80 = bias via ones-row
    C2 = np.zeros((CW, DOUT), dtype=f)
    C2[8:16] = shared_B.T * (cw * SCALING)
    C2[16:80] = expert_B.transpose(0, 2, 1).reshape(E * R, DOUT)
    C2[80] = base_b
    C2 = C2.astype(BF16)

    # scale[j] = sum_e Emap[e, j] * r[e]:
    #   taskemb rows -> 0, shared rows -> 1 (softmax sums to 1),
    #   expert row (e,r) -> (1-cw)*SCALING*r_e
    Emap = np.zeros((E, AW), dtype=f)
    Emap[:, 8:16] = 1.0
    for e in range(E):
        Emap[e, 16 + 8 * e : 24 + 8 * e] = (1.0 - cw) * SCALING
    Emap = Emap.astype(BF16)

    ones = np.ones((M_CORE,), dtype=BF16)

    in_maps = []
    for c in range(N_CORES):
        xT = np.ascontiguousarray(x[c * M_CORE : (c + 1) * M_CORE].T).astype(BF16)
        in_maps.append(
            {"xT": xT, "WT": WT, "AallT": AallT, "C2": C2, "Emap": Emap,
             "ones": ones}
        )
    return in_maps


def kernel(**inputs):
    global _cached, LAST_RESULT
    if _cached is None:
        _cached = _build_nc()
    nc = _cached
    in_maps = _prep_inputs(**inputs)
    res = run_bass_kernel_spmd(
        nc, in_maps, core_ids=list(range(N_CORES)), trace=TRACE
    )
    LAST_RESULT = res
    out = np.concatenate(
        [res.results[c]["out"] for c in range(N_CORES)], axis=0
    ).reshape(B, S, DOUT)
    return np.ascontiguousarray(out.astype(np.float32))
